# revision 14
# baseline (speedup 1.0000x reference)
"""Capsule-routing kernel for Trainium2 (8 NeuronCores, data-parallel over batch).

Reference (per item, S=512 input caps, N=32 output caps, D=64, 3 iters):
    u_hat = (u @ W).reshape(S, N, D)        # never materialized
    b = 0
    for it in 0..2:
        c = softmax(b, axis=caps)
        o = squash(einsum('ns,nsd->nd', c, u_hat))   # squash = L2 normalize
        if it < 2: b = einsum('nd,nsd->ns', o, u_hat)

Re-association (per item):
    mT[i, n] = sum_s u[s,i] c[n,s]            (m-step, contract s)
    o[n, d]  = sum_i mT[i,n] W[i, n*64+d]     (o-step, block-diag, contract i)
    P[i, n]  = sum_d W[i, n*64+d] o[n,d]      (P-step, contract d via W^T)
    r[n, s]  = sum_i P[i,n] u[s,i]            (b-step, contract i via u^T)

Everything runs in bf16 on the PE (fp32 PSUM accumulate).

Schedule (cost-model informed):
  - All input DMAs are SWDGE cast-loads (fp32->bf16): DMA cost is out-bytes /
    360 GB/s, so casting halves it. Only ~5 SWDGE completion sems are usable
    before a later DMA's desc gen stalls, so: W + 4 two-item u chunks = 5.
  - NO DMA transposes (they serialize with the loads on the shared DMA
    engines; the 50us version lost ~7us there). All W^T/u^T via PE
    transposes; the PSUM->SBUF copies alternate DVE/Act.
  - Engines execute in order, so EMISSION ORDER IS THE SCHEDULE: everything
    is emitted in data-arrival order (W^T build; per-pair u^T + it0 m as
    each 2-item chunk lands; group A it0; pairs 45/67 + group B it0; then
    A1, B1, A2, B2). Two groups of 4 items pipeline ~2 iterations deep.
  - Squash avoids Act's expensive Square+accum: Pool squares o^T in SBUF
    (bf16), PE row-reduces the squares -> n2, Act does Ln/Exp (rsqrt),
    DVE applies the scale.
"""

import sys

import numpy as np

if "/opt/trn_rl_repo" not in sys.path:
    sys.path.insert(0, "/opt/trn_rl_repo")

import concourse.bass as bass  # noqa: F401
import concourse.mybir as mybir
import concourse.tile as tile
from concourse import bacc
from concourse.masks import make_identity

# Keep Exp/Ln/Square/Copy resolvable via one activation table so the kernel
# needs a single LoadActFuncSet (table swaps cost ~1.3us each).
_orig_get_tables = bacc.get_activation_tables


def _tables_prefer_nle(arch):
    t = _orig_get_tables(arch)
    pref = "natural_log_exp_and_others"
    if pref not in t:
        return t
    mine = t[pref]
    return {k: (v if k == pref else v - mine) for k, v in t.items()}


bacc.get_activation_tables = _tables_prefer_nle

FP = mybir.dt.float32
BF = mybir.dt.bfloat16
EPS = 1e-7
B, S, I = 64, 512, 512          # full batch, input caps, input dim
N, D = 32, 64                   # output caps, cap dim
NCORES = 8
BC = B // NCORES                # items per core = 8
G = 2                           # routing groups
BG = BC // G                    # items per group = 4
P = 128
IC = I // P                     # i chunks = 4
J = 4                           # s = 4*p + j
ROUTINGS = 3
ND = N * D


def _ap(base, offset_delta, dims):
    return bass.AP(tensor=base.tensor, offset=base.offset + offset_delta,
                   ap=dims)


def _copy(eng, nc, dst, src):
    if eng is nc.scalar:
        eng.copy(dst, src)
    else:
        eng.tensor_copy(dst, src)


def build_kernel(nc):
    u_dram = nc.dram_tensor("u", [BC, S, I], FP, kind="ExternalInput").ap()
    w_dram = nc.dram_tensor("W", [I, ND], FP, kind="ExternalInput").ap()
    o_dram = nc.dram_tensor("out", [BC, N, D], FP, kind="ExternalOutput").ap()

    with tile.TileContext(nc) as tc:
        _body(tc, u_dram, w_dram, o_dram)
    return nc


class _Ctx:
    pass


def _body(tc, u_dram, w_dram, o_dram):
    from contextlib import ExitStack

    nc = tc.nc
    ctx = ExitStack()
    with ctx:
        statics = ctx.enter_context(tc.tile_pool(name="statics", bufs=1))
        stage = ctx.enter_context(tc.tile_pool(name="stage", bufs=3))
        psum = ctx.enter_context(tc.tile_pool(name="psum", bufs=2, space="PSUM"))

        c = _Ctx()
        c.stage, c.psum, c.o_dram = stage, psum, o_dram

        # ---------- statics ----------
        ident_f = statics.tile([P, P], FP)
        make_identity(nc, ident_f)
        c.ident = statics.tile([P, P], BF)
        nc.vector.tensor_copy(c.ident, ident_f)
        c.eps_sb = statics.tile([P, 1], FP)
        nc.vector.memset(c.eps_sb, EPS)

        c.w_bf = statics.tile([P, IC, ND], BF)     # W[128*ic+p, nd]
        c.wt = statics.tile([P, N // 2, I], BF)    # W[i, 128*q+p] at [p, q, i]
        c.u_bf = statics.tile([P, BC, J, I], BF)   # u[b, 4p+j, i]
        c.ut = statics.tile([P, BC, J * IC, P], BF)  # u[b,4q+j,128ic+v] at [v,b,4j+ic,q]
        c.ct = statics.tile([P, BC, J, N], BF)     # c[b, n, 4p+j]
        c.mt = statics.tile([P, IC, N, BC], BF)    # m[b, n, 128ic+v] at [v, ic, n, b]
        c.bd = [statics.tile([P, N // 2, 2 * BG], BF, name=f"bd{g}")
                for g in range(G)]                 # o[4g+bi, 2q+h, d] at [64h+d, q, 4h+bi]

        c.ones_n = statics.tile([P, 1], BF)
        nc.vector.memset(c.ones_n, 1.0 / N)
        c.ones64 = statics.tile([P, 1], BF)
        nc.vector.memset(c.ones64, 1.0)
        for g in range(G):
            nc.vector.memset(c.bd[g].rearrange("p a b -> p (a b)"), 0.0)

        # ---------- cast-loads (SWDGE converts fp32 -> bf16 in the DMA) ----
        with tc.high_priority(offset=-2000):
            nc.gpsimd.dma_start(
                out=c.w_bf, in_=w_dram.rearrange("(c p) n -> p c n", p=P))
            for ch in range(BC // 2):
                nc.gpsimd.dma_start(
                    out=c.u_bf[:, 2 * ch:2 * ch + 2],
                    in_=u_dram[2 * ch:2 * ch + 2].rearrange(
                        "b (p j) i -> p b j i", j=J))

        # ---------- W^T build (PE transposes, copies DVE/Act alternating) --
        for qq in range(N // 4):
            tbw = psum.tile([P, 1024], BF, tag="tp", name="tbw", bufs=3)
            for dq in range(2):
                q = 2 * qq + dq
                for ic in range(IC):
                    nc.tensor.transpose(
                        tbw[:, (dq * IC + ic) * P:(dq * IC + ic + 1) * P],
                        c.w_bf[:, ic, q * P:(q + 1) * P], c.ident)
            eng = nc.vector if qq % 2 == 0 else nc.scalar
            _copy(eng, nc, c.wt[:, 2 * qq:2 * qq + 2, :], tbw)

        # ---------- routing, emitted in data-arrival order ----------
        # pairs 0,1 land, then 2,3 -> group A it0; pairs 4..7 -> B it0; ...
        _pair_prep(tc, c, 0)
        _pair_prep(tc, c, 1)
        _route_iter(tc, c, it=0, g=0)
        _pair_prep(tc, c, 2)
        _pair_prep(tc, c, 3)
        _route_iter(tc, c, it=0, g=1)
        _m_step(tc, c, it=1, g=0)
        _route_iter(tc, c, it=1, g=0)
        _m_step(tc, c, it=1, g=1)
        _route_iter(tc, c, it=1, g=1)
        _m_step(tc, c, it=2, g=0)
        _fill(tc, c, 20)
        _route_iter(tc, c, it=2, g=0)
        _m_step(tc, c, it=2, g=1)
        _fill(tc, c, 16)
        _route_iter(tc, c, it=2, g=1)


def _fill(tc, c, k):
    # PE p-state warmers: junk transposes in natural PE idle windows so the
    # final iteration's matmuls run at full clock.
    nc = tc.nc
    junk = c.psum.tile([P, 1024], BF, tag="tp", name="junk", bufs=3)
    for _ in range(k):
        nc.tensor.transpose(junk[:, 0:P], c.ident, c.ident)


def _pair_prep(tc, c, pair):
    """u^T transposes + it0 m-step for items 2*pair, 2*pair+1 (one u chunk)."""
    nc = tc.nc
    for b in (2 * pair, 2 * pair + 1):
        for half in range(2):
            tbu = c.psum.tile([P, 1024], BF, tag="tp", name="tbu", bufs=3)
            for jj in range(2):
                j = 2 * half + jj
                for ic in range(IC):
                    nc.tensor.transpose(
                        tbu[:, (jj * IC + ic) * P:(jj * IC + ic + 1) * P],
                        c.u_bf[:, b, j, ic * P:(ic + 1) * P], c.ident)
            eng = nc.vector if (b + half) % 2 == 0 else nc.scalar
            _copy(eng, nc, c.ut[:, b, half * 2 * IC:(half + 1) * 2 * IC, :],
                  tbu)
    # it0 m-step: c == 1/N exactly -> m0[i] = (1/N) sum_s u[s,i], same for
    # every cap: one PSUM column per (item, i-chunk) via a ones-vector.
    pm = c.psum.tile([P, 2, IC], FP, tag="pm", name="pm0", bufs=1)
    for k, b in enumerate((2 * pair, 2 * pair + 1)):
        for ic in range(IC):
            for j in range(J):
                nc.tensor.matmul(
                    pm[:, k, ic:ic + 1],
                    lhsT=c.u_bf[:, b, j, ic * P:(ic + 1) * P],
                    rhs=c.ones_n,
                    start=(j == 0), stop=(j == J - 1))
    nc.vector.tensor_copy(
        c.mt[:, :, 0, 2 * pair:2 * pair + 2],
        pm.rearrange("p b i -> p i b"))


def _m_step(tc, c, it, g):
    """m-step for iterations >= 1: contract s with the softmaxed c^T."""
    nc = tc.nc
    pm = c.psum.tile([P, BG, IC, N], FP, tag="pm", name="pm", bufs=1)
    for bi in range(BG):
        b = BG * g + bi
        for ic in range(IC):
            for j in range(J):
                nc.tensor.matmul(
                    pm[:, bi, ic, :],
                    lhsT=c.u_bf[:, b, j, ic * P:(ic + 1) * P],
                    rhs=c.ct[:, b, j, :],
                    start=(j == 0), stop=(j == J - 1))
    nc.vector.tensor_copy(c.mt[:, :, :, BG * g:BG * (g + 1)],
                          pm.rearrange("p b i n -> p i n b"))


def _route_iter(tc, c, it, g):
    """o-step -> squash -> (P-step -> b-step -> softmax | output DMA)."""
    nc = tc.nc
    stage, psum = c.stage, c.psum
    last = it == ROUTINGS - 1

    # squash bank: ot (fp32, o-step dst) | on (bf16) | ots (bf16) | n2 (fp32)
    # carved from one 2KB PSUM bank.
    sqb = psum.tile([P, 1024], BF, tag="sq", name="sqb", bufs=2)
    ot = sqb.bitcast(FP)[0:D, 0:N * BG]
    on_ps = sqb[:, 256:256 + D]
    oTs = sqb[0:D, 384:384 + P]
    n2p = sqb.bitcast(FP)[:, 320:321]

    # o-step: ot[d, 4n+bi] = sum_i mT[i,n] W[i, n*64+d] for the group
    for n in range(N):
        for ic in range(IC):
            n_src = 0 if it == 0 else n
            nc.tensor.matmul(
                ot[:, n * BG:(n + 1) * BG],
                lhsT=c.w_bf[:, ic, n * D:(n + 1) * D],
                rhs=c.mt[:, ic, n_src, BG * g:BG * (g + 1)],
                start=(ic == 0), stop=(ic == IC - 1))

    # squash: one Act copy of o^T to SBUF, then in parallel
    #   PE transposes it to (n,b)-on-partitions   (for the scale apply)
    #   Pool squares it (SBUF) + PE row-reduce    (norm^2 per (n,b))
    oTu = stage.tile([D, N * BG], BF, tag="oTu", name="oTu")
    nc.scalar.copy(oTu, ot)
    nc.tensor.transpose(on_ps, oTu, c.ident[:D, :D])
    sq = stage.tile([D, N * BG], BF, tag="sqs", name="sqs")
    nc.gpsimd.tensor_tensor(sq, oTu, oTu, mybir.AluOpType.mult)
    nc.tensor.matmul(n2p, lhsT=sq, rhs=c.ones64[0:D], start=True, stop=True)
    lg = stage.tile([P, 1], FP, tag="lg", name="lg")
    nc.scalar.activation(lg, n2p, mybir.ActivationFunctionType.Ln,
                         bias=c.eps_sb[:, 0:1])
    rs = stage.tile([P, 1], FP, tag="rs", name="rs")
    nc.scalar.activation(rs, lg, mybir.ActivationFunctionType.Exp, scale=-0.5)
    rs_b = bass.AP(tensor=rs.tensor, offset=rs.offset, ap=[rs.ap[0], [0, D]])

    if last:
        onf = stage.tile([P, D], FP, tag="onf", name="onf")
        nc.scalar.activation(onf, on_ps, mybir.ActivationFunctionType.Copy,
                             scale=rs[:, 0:1])
        dst = _ap(c.o_dram, BG * g * N * D, [[D, N], [N * D, BG], [1, D]])
        nc.sync.dma_start(out=dst, in_=onf)
        return

    # scale on DVE (rs broadcast along free dim), transpose back, block-diag
    onb = stage.tile([P, D], BF, tag="onb", name="onb")
    nc.vector.tensor_tensor(onb, on_ps, rs_b, mybir.AluOpType.mult)
    nc.tensor.transpose(oTs, onb, c.ident)
    oTs_v = oTs.rearrange("p (q x) -> p q x", q=N // 2)
    nc.vector.tensor_copy(c.bd[g][0:D, :, 0:BG], oTs_v[:, :, 0:BG])
    nc.vector.tensor_copy(c.bd[g][D:P, :, BG:2 * BG], oTs_v[:, :, BG:2 * BG])

    # P-step: P[i, (h,bi)] per pair q; contract (h,d) on partitions
    pp = psum.tile([P, IC, N // 2, 2 * BG], FP, tag="pp", name="pp", bufs=1)
    for ic in range(IC):
        for q in range(N // 2):
            nc.tensor.matmul(
                pp[:, ic, q, :],
                lhsT=c.wt[:, q, ic * P:(ic + 1) * P],
                rhs=c.bd[g][:, q, :],
                start=True, stop=True)
    pt = stage.tile([P, IC, N // 2, 2 * BG], BF, tag="pt", name="pt")
    nc.scalar.copy(pt, pp)

    # b-step + per-pair softmax (the next iteration's m-step starts on the
    # first pair while the second is still normalizing). The first pair's
    # normalize-multiply goes to Pool (off the critical chain); the second
    # pair's stays on DVE.
    rt = psum.tile([P, BG, J, N], FP, tag="rt", name="rt", bufs=1)
    for bi in range(BG):
        b = BG * g + bi
        for j in range(J):
            for ic in range(IC):
                rhs = _ap(pt, ic * (N // 2) * 2 * BG + bi,
                          [pt.ap[0], [2 * BG, N // 2], [BG, 2]])
                nc.tensor.matmul(
                    rt[:, bi, j, :],
                    lhsT=c.ut[:, b, j * IC + ic, :],
                    rhs=rhs,
                    start=(ic == 0), stop=(ic == IC - 1))
        if bi % 2 == 1:
            b0 = b - 1
            et = stage.tile([P, 2 * J, N], FP, tag="et", name=f"et{bi}",
                            bufs=4)
            nc.scalar.activation(
                et, rt[:, bi - 1:bi + 1].rearrange("p b j n -> p (b j) n"),
                mybir.ActivationFunctionType.Exp)
            zz = stage.tile([P, 2 * J], FP, tag="zz", name=f"zz{bi}", bufs=4)
            nc.vector.reduce_sum(zz, et, axis=mybir.AxisListType.X)
            rz = stage.tile([P, 2 * J], FP, tag="rz", name=f"rz{bi}", bufs=4)
            nc.vector.reciprocal(rz, zz)
            rz_b = bass.AP(tensor=rz.tensor, offset=rz.offset,
                           ap=[rz.ap[0], [1, 2 * J], [0, N]])
            eng = nc.gpsimd if bi == 1 else nc.vector
            eng.tensor_tensor(
                c.ct[:, b0:b0 + 2].rearrange("p b j n -> p (b j) n"),
                et, rz_b, mybir.AluOpType.mult)


_COMPILED = None


def _get_compiled():
    global _COMPILED
    if _COMPILED is None:
        nc = bacc.Bacc("TRN2", target_bir_lowering=False, debug=False,
                       num_devices=NCORES,
                       dynamic_dma_scratch_size=49152)
        build_kernel(nc)
        nc.compile()
        _COMPILED = nc
    return _COMPILED


def kernel(u_vecs, W):
    from concourse.bass_utils import run_bass_kernel_spmd

    u_vecs = np.ascontiguousarray(u_vecs, dtype=np.float32)
    W = np.ascontiguousarray(W, dtype=np.float32)
    assert u_vecs.shape == (B, S, I) and W.shape == (I, ND)

    nc = _get_compiled()
    in_maps = [
        {"u": u_vecs[c * BC:(c + 1) * BC], "W": W} for c in range(NCORES)
    ]
    res = run_bass_kernel_spmd(nc, in_maps, list(range(NCORES)))
    return np.concatenate(
        [res.results[c]["out"] for c in range(NCORES)], axis=0
    ).astype(np.float32)


# revision 18
# speedup vs baseline: 1.0412x; 1.0412x over previous
"""Capsule-routing kernel for Trainium2 (8 NeuronCores, data-parallel over batch).

Reference (per item, S=512 input caps, N=32 output caps, D=64, 3 iters):
    u_hat = (u @ W).reshape(S, N, D)        # never materialized
    b = 0
    for it in 0..2:
        c = softmax(b, axis=caps)
        o = squash(einsum('ns,nsd->nd', c, u_hat))   # squash = L2 normalize
        if it < 2: b = einsum('nd,nsd->ns', o, u_hat)

Re-association (per item):
    mT[i, n] = sum_s u[s,i] c[n,s]            (m-step, contract s)
    o[n, d]  = sum_i mT[i,n] W[i, n*64+d]     (o-step, block-diag, contract i)
    P[i, n]  = sum_d W[i, n*64+d] o[n,d]      (P-step, contract d via W^T)
    r[n, s]  = sum_i P[i,n] u[s,i]            (b-step, contract i via u^T)

Everything runs in bf16 on the PE (fp32 PSUM accumulate).

Schedule (cost-model informed):
  - All input DMAs are SWDGE cast-loads (fp32->bf16): DMA cost is out-bytes /
    360 GB/s, so casting halves it. Only ~5 SWDGE completion sems are usable
    before a later DMA's desc gen stalls, so: W + 4 two-item u chunks = 5.
  - NO DMA transposes (they serialize with the loads on the shared DMA
    engines; the 50us version lost ~7us there). All W^T/u^T via PE
    transposes; the PSUM->SBUF copies alternate DVE/Act.
  - Engines execute in order, so EMISSION ORDER IS THE SCHEDULE: everything
    is emitted in data-arrival order (W^T build; per-pair u^T + it0 m as
    each 2-item chunk lands; group A it0; pairs 45/67 + group B it0; then
    A1, B1, A2, B2). Two groups of 4 items pipeline ~2 iterations deep.
  - Squash avoids Act's expensive Square+accum: Pool squares o^T in SBUF
    (bf16), PE row-reduces the squares -> n2, Act does Ln/Exp (rsqrt),
    DVE applies the scale.
"""

import sys

import numpy as np

if "/opt/trn_rl_repo" not in sys.path:
    sys.path.insert(0, "/opt/trn_rl_repo")

import concourse.bass as bass  # noqa: F401
import concourse.mybir as mybir
import concourse.tile as tile
from concourse import bacc
from concourse.masks import make_identity

# Keep Exp/Ln/Square/Copy resolvable via one activation table so the kernel
# needs a single LoadActFuncSet (table swaps cost ~1.3us each).
_orig_get_tables = bacc.get_activation_tables


def _tables_prefer_nle(arch):
    t = _orig_get_tables(arch)
    pref = "natural_log_exp_and_others"
    if pref not in t:
        return t
    mine = t[pref]
    return {k: (v if k == pref else v - mine) for k, v in t.items()}


bacc.get_activation_tables = _tables_prefer_nle

FP = mybir.dt.float32
BF = mybir.dt.bfloat16
EPS = 1e-7
B, S, I = 64, 512, 512          # full batch, input caps, input dim
N, D = 32, 64                   # output caps, cap dim
NCORES = 8
BC = B // NCORES                # items per core = 8
G = 2                           # routing groups
BG = BC // G                    # items per group = 4
P = 128
IC = I // P                     # i chunks = 4
J = 4                           # s = 4*p + j
ROUTINGS = 3
ND = N * D


def _ap(base, offset_delta, dims):
    return bass.AP(tensor=base.tensor, offset=base.offset + offset_delta,
                   ap=dims)


def _copy(eng, nc, dst, src):
    if eng is nc.scalar:
        eng.copy(dst, src)
    else:
        eng.tensor_copy(dst, src)


def build_kernel(nc):
    u_dram = nc.dram_tensor("u", [BC, S, I], FP, kind="ExternalInput").ap()
    w_dram = nc.dram_tensor("W", [I, ND], FP, kind="ExternalInput").ap()
    o_dram = nc.dram_tensor("out", [BC, N, D], FP, kind="ExternalOutput").ap()

    with tile.TileContext(nc) as tc:
        _body(tc, u_dram, w_dram, o_dram)
    return nc


class _Ctx:
    pass


def _body(tc, u_dram, w_dram, o_dram):
    from contextlib import ExitStack

    nc = tc.nc
    ctx = ExitStack()
    with ctx:
        statics = ctx.enter_context(tc.tile_pool(name="statics", bufs=1))
        stage = ctx.enter_context(tc.tile_pool(name="stage", bufs=3))
        psum = ctx.enter_context(tc.tile_pool(name="psum", bufs=2, space="PSUM"))

        c = _Ctx()
        c.stage, c.psum, c.o_dram = stage, psum, o_dram

        # ---------- statics ----------
        ident_f = statics.tile([P, P], FP)
        make_identity(nc, ident_f)
        c.ident = statics.tile([P, P], BF)
        nc.vector.tensor_copy(c.ident, ident_f)
        c.eps_sb = statics.tile([P, 1], FP)
        nc.vector.memset(c.eps_sb, EPS)

        c.w_bf = statics.tile([P, IC, ND], BF)     # W[128*ic+p, nd]
        c.wt = statics.tile([P, N // 2, I], BF)    # W[i, 128*q+p] at [p, q, i]
        c.u_bf = statics.tile([P, BC, J, I], BF)   # u[b, 4p+j, i]
        c.ut = statics.tile([P, BC, J * IC, P], BF)  # u[b,4q+j,128ic+v] at [v,b,4j+ic,q]
        c.ct = statics.tile([P, BC, J, N], BF)     # c[b, n, 4p+j]
        c.mt = statics.tile([P, IC, N, BC], BF)    # m[b, n, 128ic+v] at [v, ic, n, b]
        c.bd = [statics.tile([P, N // 2, 2 * BG], BF, name=f"bd{g}")
                for g in range(G)]                 # o[4g+bi, 2q+h, d] at [64h+d, q, 4h+bi]

        c.ones_n = statics.tile([P, 1], BF)
        nc.vector.memset(c.ones_n, 1.0 / N)
        c.ones64 = statics.tile([P, 1], BF)
        nc.vector.memset(c.ones64, 1.0)
        for g in range(G):
            nc.vector.memset(c.bd[g].rearrange("p a b -> p (a b)"), 0.0)

        # ---------- cast-loads (SWDGE converts fp32 -> bf16 in the DMA) ----
        with tc.high_priority(offset=-2000):
            nc.gpsimd.dma_start(
                out=c.w_bf, in_=w_dram.rearrange("(c p) n -> p c n", p=P))
            for ch in range(BC // 2):
                nc.gpsimd.dma_start(
                    out=c.u_bf[:, 2 * ch:2 * ch + 2],
                    in_=u_dram[2 * ch:2 * ch + 2].rearrange(
                        "b (p j) i -> p b j i", j=J))

        # ---------- W^T build (PE transposes, copies DVE/Act alternating) --
        for qq in range(N // 4):
            tbw = psum.tile([P, 1024], BF, tag="tp", name="tbw", bufs=3)
            for dq in range(2):
                q = 2 * qq + dq
                for ic in range(IC):
                    nc.tensor.transpose(
                        tbw[:, (dq * IC + ic) * P:(dq * IC + ic + 1) * P],
                        c.w_bf[:, ic, q * P:(q + 1) * P], c.ident)
            eng = nc.vector if qq % 2 == 0 else nc.scalar
            _copy(eng, nc, c.wt[:, 2 * qq:2 * qq + 2, :], tbw)

        # ---------- routing, emitted in data-arrival order ----------
        # pairs 0,1 land, then 2,3 -> group A it0; pairs 4..7 -> B it0; ...
        _pair_prep(tc, c, 0)
        _pair_prep(tc, c, 1)
        _route_iter(tc, c, it=0, g=0)
        _pair_prep(tc, c, 2)
        _pair_prep(tc, c, 3, ut_via_dma=True)
        _route_iter(tc, c, it=0, g=1)
        _m_step(tc, c, it=1, g=0)
        _route_iter(tc, c, it=1, g=0)
        _m_step(tc, c, it=1, g=1)
        _route_iter(tc, c, it=1, g=1)
        _m_step(tc, c, it=2, g=0)
        _fill(tc, c, 20)
        _route_iter(tc, c, it=2, g=0)
        _m_step(tc, c, it=2, g=1)
        _fill(tc, c, 16)
        _route_iter(tc, c, it=2, g=1)


def _fill(tc, c, k):
    # PE p-state warmers: junk transposes in natural PE idle windows so the
    # final iteration's matmuls run at full clock.
    nc = tc.nc
    junk = c.psum.tile([P, 1024], BF, tag="tp", name="junk", bufs=3)
    for _ in range(k):
        nc.tensor.transpose(junk[:, 0:P], c.ident, c.ident)


def _pair_prep(tc, c, pair, ut_via_dma=False):
    """u^T transposes + it0 m-step for items 2*pair, 2*pair+1 (one u chunk).

    ut_via_dma: build u^T with the DMA crossbar transpose (SBUF->SBUF)
    instead of PE transposes + DVE/Act copies. The DMA engines idle once the
    input loads finish, and this keeps ~3us of copies out of the in-order
    vector queues for the items whose u^T is needed last.
    """
    nc = tc.nc
    for b in (2 * pair, 2 * pair + 1):
        if ut_via_dma:
            nc.sync.dma_start(out=c.ut[:, b], in_=c.u_bf[:, b],
                              transpose=True)
            continue
        for half in range(2):
            tbu = c.psum.tile([P, 1024], BF, tag="tp", name="tbu", bufs=3)
            for jj in range(2):
                j = 2 * half + jj
                for ic in range(IC):
                    nc.tensor.transpose(
                        tbu[:, (jj * IC + ic) * P:(jj * IC + ic + 1) * P],
                        c.u_bf[:, b, j, ic * P:(ic + 1) * P], c.ident)
            dst = c.ut[:, b, half * 2 * IC:(half + 1) * 2 * IC, :]
            eng = nc.vector if (b + half) % 2 == 0 else nc.scalar
            _copy(eng, nc, dst, tbu)
    # it0 m-step: c == 1/N exactly -> m0[i] = (1/N) sum_s u[s,i], same for
    # every cap: one PSUM column per (item, i-chunk) via a ones-vector.
    pm = c.psum.tile([P, 2, IC], FP, tag="pm", name="pm0", bufs=1)
    for k, b in enumerate((2 * pair, 2 * pair + 1)):
        for ic in range(IC):
            for j in range(J):
                nc.tensor.matmul(
                    pm[:, k, ic:ic + 1],
                    lhsT=c.u_bf[:, b, j, ic * P:(ic + 1) * P],
                    rhs=c.ones_n,
                    start=(j == 0), stop=(j == J - 1))
    nc.vector.tensor_copy(
        c.mt[:, :, 0, 2 * pair:2 * pair + 2],
        pm.rearrange("p b i -> p i b"))


def _m_step(tc, c, it, g):
    """m-step for iterations >= 1: contract s with the softmaxed c^T."""
    nc = tc.nc
    pm = c.psum.tile([P, BG, IC, N], FP, tag="pm", name="pm", bufs=1)
    for bi in range(BG):
        b = BG * g + bi
        for ic in range(IC):
            for j in range(J):
                nc.tensor.matmul(
                    pm[:, bi, ic, :],
                    lhsT=c.u_bf[:, b, j, ic * P:(ic + 1) * P],
                    rhs=c.ct[:, b, j, :],
                    start=(j == 0), stop=(j == J - 1))
    nc.vector.tensor_copy(c.mt[:, :, :, BG * g:BG * (g + 1)],
                          pm.rearrange("p b i n -> p i n b"))


def _route_iter(tc, c, it, g):
    """o-step -> squash -> (P-step -> b-step -> softmax | output DMA)."""
    nc = tc.nc
    stage, psum = c.stage, c.psum
    last = it == ROUTINGS - 1

    # squash bank: ot (fp32, o-step dst) | on (bf16) | ots (bf16) | n2 (fp32)
    # carved from one 2KB PSUM bank.
    sqb = psum.tile([P, 1024], BF, tag="sq", name="sqb", bufs=2)
    ot = sqb.bitcast(FP)[0:D, 0:N * BG]
    on_ps = sqb[:, 256:256 + D]
    oTs = sqb[0:D, 384:384 + P]
    n2p = sqb.bitcast(FP)[:, 320:321]

    # o-step: ot[d, 4n+bi] = sum_i mT[i,n] W[i, n*64+d] for the group
    for n in range(N):
        for ic in range(IC):
            n_src = 0 if it == 0 else n
            nc.tensor.matmul(
                ot[:, n * BG:(n + 1) * BG],
                lhsT=c.w_bf[:, ic, n * D:(n + 1) * D],
                rhs=c.mt[:, ic, n_src, BG * g:BG * (g + 1)],
                start=(ic == 0), stop=(ic == IC - 1))

    # squash: one Act copy of o^T to SBUF, then in parallel
    #   PE transposes it to (n,b)-on-partitions   (for the scale apply)
    #   Pool squares it (SBUF) + PE row-reduce    (norm^2 per (n,b))
    oTu = stage.tile([D, N * BG], BF, tag="oTu", name="oTu")
    nc.scalar.copy(oTu, ot)
    nc.tensor.transpose(on_ps, oTu, c.ident[:D, :D])
    sq = stage.tile([D, N * BG], BF, tag="sqs", name="sqs")
    nc.gpsimd.tensor_tensor(sq, oTu, oTu, mybir.AluOpType.mult)
    nc.tensor.matmul(n2p, lhsT=sq, rhs=c.ones64[0:D], start=True, stop=True)
    lg = stage.tile([P, 1], FP, tag="lg", name="lg")
    nc.scalar.activation(lg, n2p, mybir.ActivationFunctionType.Ln,
                         bias=c.eps_sb[:, 0:1])
    rs = stage.tile([P, 1], FP, tag="rs", name="rs")
    nc.scalar.activation(rs, lg, mybir.ActivationFunctionType.Exp, scale=-0.5)
    rs_b = bass.AP(tensor=rs.tensor, offset=rs.offset, ap=[rs.ap[0], [0, D]])

    if last:
        onf = stage.tile([P, D], FP, tag="onf", name="onf")
        nc.scalar.activation(onf, on_ps, mybir.ActivationFunctionType.Copy,
                             scale=rs[:, 0:1])
        dst = _ap(c.o_dram, BG * g * N * D, [[D, N], [N * D, BG], [1, D]])
        nc.sync.dma_start(out=dst, in_=onf)
        return

    # scale on DVE (rs broadcast along free dim), transpose back, block-diag
    onb = stage.tile([P, D], BF, tag="onb", name="onb")
    nc.vector.tensor_tensor(onb, on_ps, rs_b, mybir.AluOpType.mult)
    nc.tensor.transpose(oTs, onb, c.ident)
    oTs_v = oTs.rearrange("p (q x) -> p q x", q=N // 2)
    nc.vector.tensor_copy(c.bd[g][0:D, :, 0:BG], oTs_v[:, :, 0:BG])
    nc.vector.tensor_copy(c.bd[g][D:P, :, BG:2 * BG], oTs_v[:, :, BG:2 * BG])

    # P-step: P[i, (h,bi)] per pair q; contract (h,d) on partitions
    pp = psum.tile([P, IC, N // 2, 2 * BG], FP, tag="pp", name="pp", bufs=1)
    for ic in range(IC):
        for q in range(N // 2):
            nc.tensor.matmul(
                pp[:, ic, q, :],
                lhsT=c.wt[:, q, ic * P:(ic + 1) * P],
                rhs=c.bd[g][:, q, :],
                start=True, stop=True)
    pt = stage.tile([P, IC, N // 2, 2 * BG], BF, tag="pt", name="pt")
    nc.scalar.copy(pt, pp)

    # b-step + per-pair softmax (the next iteration's m-step starts on the
    # first pair while the second is still normalizing). The first pair's
    # normalize-multiply goes to Pool (off the critical chain); the second
    # pair's stays on DVE.
    rt = psum.tile([P, BG, J, N], FP, tag="rt", name="rt", bufs=1)
    for bi in range(BG):
        b = BG * g + bi
        for j in range(J):
            for ic in range(IC):
                rhs = _ap(pt, ic * (N // 2) * 2 * BG + bi,
                          [pt.ap[0], [2 * BG, N // 2], [BG, 2]])
                nc.tensor.matmul(
                    rt[:, bi, j, :],
                    lhsT=c.ut[:, b, j * IC + ic, :],
                    rhs=rhs,
                    start=(ic == 0), stop=(ic == IC - 1))
        if bi % 2 == 1:
            b0 = b - 1
            et = stage.tile([P, 2 * J, N], FP, tag="et", name=f"et{bi}",
                            bufs=4)
            nc.scalar.activation(
                et, rt[:, bi - 1:bi + 1].rearrange("p b j n -> p (b j) n"),
                mybir.ActivationFunctionType.Exp)
            zz = stage.tile([P, 2 * J], FP, tag="zz", name=f"zz{bi}", bufs=4)
            nc.vector.reduce_sum(zz, et, axis=mybir.AxisListType.X)
            rz = stage.tile([P, 2 * J], FP, tag="rz", name=f"rz{bi}", bufs=4)
            nc.vector.reciprocal(rz, zz)
            rz_b = bass.AP(tensor=rz.tensor, offset=rz.offset,
                           ap=[rz.ap[0], [1, 2 * J], [0, N]])
            eng = nc.gpsimd if bi == 1 else nc.vector
            eng.tensor_tensor(
                c.ct[:, b0:b0 + 2].rearrange("p b j n -> p (b j) n"),
                et, rz_b, mybir.AluOpType.mult)


_COMPILED = None


def _get_compiled():
    global _COMPILED
    if _COMPILED is None:
        nc = bacc.Bacc("TRN2", target_bir_lowering=False, debug=False,
                       num_devices=NCORES,
                       dynamic_dma_scratch_size=49152)
        build_kernel(nc)
        nc.compile()
        _COMPILED = nc
    return _COMPILED


def kernel(u_vecs, W):
    from concourse.bass_utils import run_bass_kernel_spmd

    u_vecs = np.ascontiguousarray(u_vecs, dtype=np.float32)
    W = np.ascontiguousarray(W, dtype=np.float32)
    assert u_vecs.shape == (B, S, I) and W.shape == (I, ND)

    nc = _get_compiled()
    in_maps = [
        {"u": u_vecs[c * BC:(c + 1) * BC], "W": W} for c in range(NCORES)
    ]
    res = run_bass_kernel_spmd(nc, in_maps, list(range(NCORES)))
    return np.concatenate(
        [res.results[c]["out"] for c in range(NCORES)], axis=0
    ).astype(np.float32)
